# revision 22
# baseline (speedup 1.0000x reference)
"""CrossModalAttention TRN2 kernel.

Strategy (data-parallel over batch, one batch element per NeuronCore):
  dir a: q from rgb, k/v from pl;  dir b: q from pl, k/v from rgb.
  bf16 datapath (SBUF), fp32 PSUM accumulation.
  Key identity: the K-projection bias cancels in softmax over keys
  (it adds a per-query constant to every score), so K needs no bias.
  Per direction:
    Q  = scale*Wq @ f_q + scale*bq    [128 e, N]  (scale folded on host)
    K  = Wk @ f_k                     [128 e, N]  (no bias)
    VT = (Wv @ f_k)^T                 [N k, 128 e] (v-bias folded into BN shift)
    per q-tile (512 wide):
      S^T_j = K_j^T @ Q_tile          [128 k, 512 q]  per k-chunk j (PSUM f32)
      E_j   = exp(S^T_j)              (ScalarE, bf16 out)
      OT   += VT_j^T @ E_j            [128 e, 512 q]  (PSUM accumulate over j)
      dn    = ones^T @ octsum(E)      [1, 512 q]  (DVE pair/quad/oct tree ->
                                       only 4 denominator matmuls per tile)
      OT_norm = OT * bcast(1/dn)      (recip_approx_fast; bcast via rank-1 MM)
  y = Wp_a @ OT_a + Wp_b @ OT_b ; out = relu(inv*y + shift)  (BN folded, incl.
  v-bias contribution: shift' = beta - mean*inv + inv*(Wp_a@bv_a + Wp_b@bv_b))

Scheduling: PE is the in-order bottleneck engine, so everything that
depends on slow DVE/ACT chains is lag-scheduled: denominator matmuls run
2 groups after their tree-sum is emitted, and each segment's normalize/
projection tail is spilled into the next segment's early groups (hooks).
"""

import sys

sys.path.insert(0, "/opt/trn_rl_repo")

import numpy as np

B = 8
C = 256
E = 128
OUT = 256
H = W = 64
N = H * W
QW = 512
SCALE = float(E) ** -0.5

_CACHE = {}


def _patch_tail_drain(tile_mod, mybir):
    # This walrus build encodes Drain as CTRL_NO_STRUCT with a single
    # sync-wait slot; split the TileContext tail drain's waits across
    # one drain instruction per semaphore.
    if getattr(tile_mod.TileContext, "_drain_patched", False):
        return
    from concourse.vector_clock import ScopedClock

    def _drain_and_barrier(self, tick_clock, wait_clock):
        nc = self.nc
        drain_inst = nc.sync.drain()
        wait_clock.add_sem_waits(
            drain_inst.ins, ScopedClock({None: tick_clock.global_clock})
        )
        si = drain_inst.ins.sync_info
        if si is not None and si.on_wait and len(si.on_wait) > 1:
            waits = list(si.on_wait)
            drain_inst.ins.sync_info = mybir.SyncInfo(
                on_wait=[waits[0]], on_update=list(si.on_update or [])
            )
            for w in waits[1:]:
                d2 = nc.sync.drain()
                d2.ins.sync_info = mybir.SyncInfo(on_wait=[w], on_update=[])
        nc.all_engine_barrier()
        popped = nc._tile_sem_poison_stack.pop()
        assert popped is self._sem_poison
        nc.clear_and_free_semaphores(list(self.sems.allocated().values()))
        nc.all_engine_barrier()

    tile_mod.TileContext._drain_and_barrier = _drain_and_barrier
    tile_mod.TileContext._drain_patched = True


def build_nc(n=N, debug=False):
    """Build the single-core Bass program. n = spatial size (4096 full)."""
    import concourse.bacc as bacc
    import concourse.tile as tile
    from concourse import mybir

    f32 = mybir.dt.float32
    f32r = mybir.dt.float32r
    bf16 = mybir.dt.bfloat16
    AFT = mybir.ActivationFunctionType

    gj = 2                    # k-chunks per S/exp group
    nqt = n // QW             # q tiles per direction
    nkc = n // 128            # k chunks
    ngrp = nkc // gj          # exp groups per segment

    nc = bacc.Bacc(trn_type="TRN2", target_bir_lowering=False, debug=False)

    def din(name, shape, dt_=bf16):
        return nc.dram_tensor(name, shape, dt_, kind="ExternalInput").ap()

    f_a_d = din("f_a", [C, n])        # rgb features (q-side of dir a)
    f_b_d = din("f_b", [C, n])        # pl features
    wq_a_d = din("wq_a", [C, E])      # (scale*W_q_rgb)^T
    wk_a_d = din("wk_a", [C, E])      # W_k_pl^T
    wv_a_d = din("wv_a", [C, E])      # W_v_pl^T
    wq_b_d = din("wq_b", [C, E])      # (scale*W_q_pl)^T
    wk_b_d = din("wk_b", [C, E])      # W_k_rgb^T
    wv_b_d = din("wv_b", [C, E])      # W_v_rgb^T
    wp_d = din("wp", [2 * E, OUT])    # w_proj^T
    bq_a_d = din("bq_a", [E, 1], f32)  # scale * b_q_rgb
    bq_b_d = din("bq_b", [E, 1], f32)  # scale * b_q_pl
    inv_d = din("bn_inv", [OUT, 1], f32)
    shf_d = din("bn_shf", [OUT, 1], f32)
    ones_c_d = din("ones_c", [E, 1], f32)   # f32r for dn matmul
    ones_r_d = din("ones_r", [1, E], f32)   # f32r for bcast matmul
    ident_d = din("ident", [E, E], f32)     # f32r for PE transposes
    y_d = nc.dram_tensor("y", [OUT, n], f32, kind="ExternalOutput").ap()

    with tile.TileContext(nc) as tc:
        with tc.tile_pool(name="const", bufs=1) as const, \
             tc.tile_pool(name="qkv", bufs=1) as qkv, \
             tc.tile_pool(name="pst", bufs=2, space="PSUM") as pst, \
             tc.tile_pool(name="pot", bufs=2, space="PSUM") as pot, \
             tc.tile_pool(name="psh", bufs=1, space="PSUM") as psh:
            # ---- constants (DMA order: critical-path first) ----
            def wload(d, nm, eng):
                t = const.tile([128, 2, E], bf16, name=nm, tag=nm)
                eng.dma_start(t[:], d.rearrange("(c p) e -> p c e", p=128))
                return t

            def vload(d, shape, nm, eng, dt_=f32):
                t = const.tile(shape, dt_, name=nm, tag=nm)
                eng.dma_start(t[:], d)
                return t

            def vload_r(d, shape, nm, eng):
                t = const.tile(shape, f32r, name=nm, tag=nm)
                eng.dma_start(t[:], d.bitcast(f32r))
                return t

            # dir-0 weights on the sync queue (feature pieces follow right
            # behind); dir-1 weights + biases issue from the still-idle
            # scalar queue, after the first cc1 feature piece.
            wq = {0: wload(wq_a_d, "wqa", nc.sync)}
            wk = {0: wload(wk_a_d, "wka", nc.sync)}
            wv = {0: wload(wv_a_d, "wva", nc.sync)}

            # ---- per-direction activations (bf16) ----
            q_sb = {d: qkv.tile([128, n], bf16, tag=f"q{d}", name=f"q_sb{d}") for d in (0, 1)}
            k_sb = {d: qkv.tile([128, n], bf16, tag=f"k{d}", name=f"k_sb{d}") for d in (0, 1)}
            vt_sb = {d: qkv.tile([128, n], bf16, tag=f"v{d}", name=f"vt_sb{d}") for d in (0, 1)}

            # 8-deep PSUM slot rotation (all 8 banks) for the projection
            # head: the per-slot eviction latency then never stalls the PE.
            class Slots:
                def __init__(self):
                    self.i = 0
                    self.cur = None

                def get(self):
                    m = self.i % 8
                    self.i += 1
                    if m < 4:
                        if m % 2 == 0:
                            self.cur = pst.tile(
                                [128, gj, QW], f32, tag="st", name="pps")
                        return (self.cur, m % 2)
                    if m < 6:
                        return (pot.tile([128, QW], f32, tag="ot", name="ppo"),
                                None)
                    if m == 6:
                        return (psh.tile([128, QW], f32, tag="sh", name="pph"),
                                None)
                    return (psh.tile([128, QW], f32, tag="dn", name="ppd"),
                            None)

            def sl(slot, s=slice(None)):
                t, h = slot
                return t[:, h, s] if h is not None else t[:, s]

            # ---- feature load + projections (feature pool freed after) ----
            with tc.tile_pool(name="feat", bufs=1) as feat:
                fsb = {
                    name: feat.tile([128, 2, n], bf16, tag=f"f{name}",
                                    name=f"f_{name}")
                    for name in ("a", "b")
                }
                npc = max(1, n // 512)   # small pieces: projections start early
                for pc in range(npc):
                    lo, hi = pc * (n // npc), (pc + 1) * (n // npc)
                    for cc in range(2):
                        for name, dd in (("a", f_a_d), ("b", f_b_d)):
                            # cc0 on sync; first cc1 piece on scalar (both
                            # HWDGE, fast issue); later cc1 pieces on gpsimd
                            # (its ~8us engine preamble + ~1us/issue is fine
                            # off the critical path)
                            if cc == 0:
                                eng = nc.sync
                            elif pc == 0:
                                eng = nc.scalar
                            else:
                                eng = nc.gpsimd
                            eng.dma_start(
                                fsb[name][:, cc, lo:hi],
                                dd[cc * 128:(cc + 1) * 128, lo:hi],
                            )
                    if pc == 0:
                        # dir-1 weights + biases behind the first cc1 piece
                        wq[1] = wload(wq_b_d, "wqb", nc.scalar)
                        wk[1] = wload(wk_b_d, "wkb", nc.scalar)
                        wv[1] = wload(wv_b_d, "wvb", nc.scalar)
                        bq = {0: vload(bq_a_d, [E, 1], "bqa", nc.scalar),
                              1: vload(bq_b_d, [E, 1], "bqb", nc.scalar)}
                # late consts (used mid/late) on gpsimd after the cc1 pieces
                wp = const.tile([128, 2, OUT], bf16, name="wp", tag="wp")
                nc.gpsimd.dma_start(
                    wp[:], wp_d.rearrange("(c p) e -> p c e", p=128))
                binv = const.tile([128, 2, 1], f32, name="binv", tag="binv")
                nc.gpsimd.dma_start(
                    binv[:], inv_d.rearrange("(c p) e -> p c e", p=128))
                bshf = const.tile([128, 2, 1], f32, name="bshf", tag="bshf")
                nc.gpsimd.dma_start(
                    bshf[:], shf_d.rearrange("(c p) e -> p c e", p=128))
                ones_c = vload_r(ones_c_d, [E, 1], "onc", nc.gpsimd)
                ones_r = vload_r(ones_r_d, [1, E], "onr", nc.gpsimd)
                ident = vload_r(ident_d, [E, E], "idt", nc.gpsimd)

                slots = Slots()
                vtmps = {
                    d: feat.tile([128, n], f32r, tag=f"vtmp{d}",
                                 name=f"vtmp{d}")
                    for d in (0, 1)
                }
                # prewarm partition_broadcast's ext-isa library (~6us IRAM
                # load) under the projection head instead of mid-attention
                pbw_s = const.tile([1, 8], f32, name="pbw_s", tag="pbw_s")
                nc.gpsimd.memset(pbw_s[:], 1.0)
                pbw_d = const.tile([128, 8], f32, name="pbw_d", tag="pbw_d")
                nc.gpsimd.partition_broadcast(pbw_d[:], pbw_s[:])
                for nt in range(nqt):
                    for d in (0, 1):
                        fq = fsb["a"] if d == 0 else fsb["b"]
                        fk = fsb["b"] if d == 0 else fsb["a"]
                        for which, wt, src_f in (
                            ("q", wq[d], fq), ("k", wk[d], fk), ("v", wv[d], fk),
                        ):
                            slot = slots.get()
                            ps = sl(slot)
                            for cc in range(2):
                                nc.tensor.matmul(
                                    ps,
                                    wt[:, cc, :],
                                    src_f[:, cc, nt * QW:(nt + 1) * QW],
                                    start=(cc == 0),
                                    stop=(cc == 1),
                                )
                            if which == "q":
                                nc.scalar.activation(
                                    q_sb[d][:, nt * QW:(nt + 1) * QW], ps,
                                    AFT.Identity, bias=bq[d][:],
                                )
                            elif which == "k":
                                nc.scalar.activation(
                                    k_sb[d][:, nt * QW:(nt + 1) * QW], ps,
                                    AFT.Copy,
                                )
                            else:
                                with nc.allow_low_precision(reason="f32r V"):
                                    nc.vector.tensor_copy(
                                        vtmps[d][:, nt * QW:(nt + 1) * QW], ps
                                    )
                # PE transposes after both dirs' projections
                for d in (0, 1):
                    for g in range(nkc // 4):
                        slot = slots.get()
                        for jj in range(4):
                            kc = 4 * g + jj
                            nc.tensor.transpose(
                                sl(slot, slice(jj * 128, (jj + 1) * 128))
                                .bitcast(f32r),
                                vtmps[d][:, kc * 128:(kc + 1) * 128],
                                ident[:],
                            )
                        with nc.allow_low_precision(reason="bf16 VT"):
                            nc.vector.tensor_copy(
                                vt_sb[d][:, g * 512:(g + 1) * 512],
                                sl(slot).bitcast(f32r),
                            )

            # ---- attention + output ----
            with tc.tile_pool(name="sex", bufs=4) as sex, \
                 tc.tile_pool(name="sred", bufs=4) as sred, \
                 tc.tile_pool(name="sot", bufs=3) as sot, \
                 tc.tile_pool(name="smisc", bufs=4) as smisc:

                def emit_S(d, qt, g):
                    """S^T matmuls for one k-chunk group -> st psum tile."""
                    qs = q_sb[d][:, qt * QW:(qt + 1) * QW]
                    st = pst.tile([128, gj, QW], f32, tag="st", name="st")
                    for jj in range(gj):
                        j = gj * g + jj
                        nc.tensor.matmul(
                            st[:, jj, :],
                            k_sb[d][:, j * 128:(j + 1) * 128],
                            qs,
                            start=True, stop=True,
                        )
                    return st

                segs = [(qt, d) for qt in range(nqt) for d in (0, 1)]
                pending = {}          # qt -> {d: osb}

                def emit_body(d, qt, st0, hooks):
                    """exp + O/dn accumulation for one (qt, d); st0 is the
                    pre-emitted group-0 S tile. hooks: group -> thunks from
                    the previous segment's tail. Returns (ot, dn, dnq)."""
                    ot = pot.tile([128, QW], f32, tag="ot", name="ot")
                    dn = psh.tile([128, QW], f32, tag="dn", name="dn")
                    st_cur = st0
                    exq = exo_prev = None
                    dnq = []          # (exoo, j) awaiting lagged dn matmul
                    for g in range(ngrp):
                        st_next = emit_S(d, qt, g + 1) if g + 1 < ngrp else None
                        ex = sex.tile([128, gj, QW], bf16, tag="ex", name="ex")
                        nc.scalar.activation(ex[:], st_cur[:], AFT.Exp)
                        # denominator tree: pair/quad (bf16) -> oct (f32r)
                        exs = sred.tile([128, QW], bf16, tag="exs", name="exs")
                        with nc.allow_low_precision(reason="bf16 denom"):
                            nc.vector.tensor_add(exs[:], ex[:, 0, :], ex[:, 1, :])
                        for jj in range(gj):
                            j = gj * g + jj
                            nc.tensor.matmul(
                                ot[:],
                                vt_sb[d][:, j * 128:(j + 1) * 128],
                                ex[:, jj, :],
                                start=(j == 0), stop=(j == nkc - 1),
                            )
                        if g % 2 == 0:
                            exq = exs
                        else:
                            exo = sred.tile([128, QW], bf16, tag="exo",
                                            name="exo")
                            with nc.allow_low_precision(reason="bf16 denom"):
                                nc.vector.tensor_add(exo[:], exq[:], exs[:])
                            if g % 4 == 1:
                                exo_prev = exo
                            else:
                                exoo = sred.tile([128, QW], f32r, tag="exoo",
                                                 name="exoo")
                                with nc.allow_low_precision(reason="f32r denom"):
                                    nc.vector.tensor_add(
                                        exoo[:], exo_prev[:], exo[:])
                                dnq.append((exoo, g // 4))
                        # lagged dn matmul: oct j runs at group 4j+5 so the
                        # DVE tree is always done before the PE reaches it
                        if dnq and dnq[0][1] * 4 + 5 <= g:
                            exoo, j = dnq.pop(0)
                            nc.tensor.matmul(
                                dn[0:1, :], ones_c[:], exoo[:],
                                start=(j == 0), stop=(j == 3),
                            )
                        for th in hooks.get(g, ()):
                            th()
                        st_cur = st_next
                    return ot, dn, dnq

                def make_tail_hooks(qt, d, ot, dn):
                    """Thunks for this segment's tail, run in the next
                    segment's early groups."""
                    rcf = smisc.tile([1, QW], f32, tag="rcf", name="rcf")
                    hold = {}

                    def t_recip():    # DVE, at the boundary right after dn3
                        nc.vector.reciprocal_approx_fast(rcf[:], dn[0:1, :])

                    def t_bc():       # GPSIMD: partition broadcast of 1/dn
                        bc_sb = smisc.tile([128, QW], f32, tag="bcs",
                                           name="bcs")
                        hold["bc"] = bc_sb
                        nc.gpsimd.partition_broadcast(bc_sb[:], rcf[:])

                    def t_mul():      # DVE: normalize
                        osb = sot.tile([128, QW], bf16, tag="osb", name="osb")
                        with nc.allow_low_precision(reason="bf16 osb"):
                            nc.vector.tensor_mul(osb[:], ot[:], hold["bc"][:])
                        pending.setdefault(qt, {})[d] = osb

                    return t_recip, t_bc, t_mul

                def make_final_hook(qt, dch):
                    def th():
                        osbs = pending[qt]
                        yp = psh.tile([128, QW], f32, tag="sh", name="yp")
                        for d2 in (0, 1):
                            nc.tensor.matmul(
                                yp[:],
                                wp[:, d2, dch * 128:(dch + 1) * 128],
                                osbs[d2][:],
                                start=(d2 == 0), stop=(d2 == 1),
                            )
                        # BN affine + relu on DVE (keeps ScalarE free for exps)
                        ya = smisc.tile([128, QW], f32, tag="ya", name="ya")
                        nc.vector.tensor_scalar(
                            ya[:], yp[:], binv[:, dch, :], bshf[:, dch, :],
                            mybir.AluOpType.mult, mybir.AluOpType.add,
                        )
                        ysb = smisc.tile([128, QW], f32, tag="ysb", name="ysb")
                        nc.vector.tensor_scalar_max(ysb[:], ya[:], 0.0)
                        nc.sync.dma_start(
                            y_d[dch * 128:(dch + 1) * 128,
                                qt * QW:(qt + 1) * QW],
                            ysb[:],
                        )
                        if dch == 1:
                            pending.pop(qt)
                    return th

                st_next0 = emit_S(segs[0][1], segs[0][0], 0)
                hooks = {}
                final_q = []     # final thunks, drained one per segment
                for i, (qt, d) in enumerate(segs):
                    ot, dn, dnq = emit_body(d, qt, st_next0, hooks)
                    last = i + 1 >= len(segs)
                    if not last:
                        nqt_, nd_ = segs[i + 1]
                        st_next0 = emit_S(nd_, nqt_, 0)
                    # close this segment's denominator: final oct matmul
                    assert len(dnq) == 1
                    exoo, j = dnq.pop(0)
                    nc.tensor.matmul(
                        dn[0:1, :], ones_c[:], exoo[:],
                        start=(j == 0), stop=(j == 3),
                    )
                    t_recip, t_bc, t_mul = make_tail_hooks(qt, d, ot, dn)
                    t_recip()
                    hooks = {1: [t_bc], 3: [t_mul]}
                    if d == 1:
                        final_q.append(make_final_hook(qt, 0))
                        final_q.append(make_final_hook(qt, 1))
                    if final_q:
                        hooks[4] = [final_q.pop(0)]
                    if last:
                        for g in sorted(hooks):
                            for th in hooks[g]:
                                th()
                        for th in final_q:
                            th()
    nc.compile()
    return nc


def _host_prep(inputs, n=N):
    import ml_dtypes
    bf = ml_dtypes.bfloat16

    f_rgb = np.ascontiguousarray(
        inputs["f_rgb"].reshape(B, C, n).astype(bf))
    f_pl = np.ascontiguousarray(
        inputs["f_pl"].reshape(B, C, n).astype(bf))

    def T(w, scale=1.0):
        return np.ascontiguousarray(
            (scale * np.asarray(w, np.float32)).T).astype(bf)

    wp = np.asarray(inputs["w_proj"], np.float32)
    inv = np.asarray(inputs["bn_gamma"], np.float32) / np.sqrt(
        np.asarray(inputs["bn_var"], np.float32) + 1e-5)
    shift = (np.asarray(inputs["bn_beta"], np.float32)
             - np.asarray(inputs["bn_mean"], np.float32) * inv
             + inv * (wp[:, :E] @ np.asarray(inputs["b_v_pl"], np.float32)
                      + wp[:, E:] @ np.asarray(inputs["b_v_rgb"], np.float32)))

    shared = {
        "wq_a": T(inputs["w_q_rgb"], SCALE),
        "wk_a": T(inputs["w_k_pl"]),
        "wv_a": T(inputs["w_v_pl"]),
        "wq_b": T(inputs["w_q_pl"], SCALE),
        "wk_b": T(inputs["w_k_rgb"]),
        "wv_b": T(inputs["w_v_rgb"]),
        "wp": T(wp),
        "bq_a": (SCALE * np.asarray(inputs["b_q_rgb"], np.float32))
        .reshape(E, 1).copy(),
        "bq_b": (SCALE * np.asarray(inputs["b_q_pl"], np.float32))
        .reshape(E, 1).copy(),
        "bn_inv": inv.reshape(OUT, 1).copy(),
        "bn_shf": shift.reshape(OUT, 1).copy(),
        "ones_c": np.ones((E, 1), np.float32),
        "ones_r": np.ones((1, E), np.float32),
        "ident": np.eye(E, dtype=np.float32),
    }
    in_maps = []
    for b in range(B):
        m = dict(shared)
        m["f_a"] = f_rgb[b]
        m["f_b"] = f_pl[b]
        in_maps.append(m)
    return in_maps


def kernel(**inputs):
    from concourse import bass_utils

    if "nc" not in _CACHE:
        _CACHE["nc"] = build_nc()
    nc = _CACHE["nc"]
    in_maps = _host_prep(inputs)
    res = bass_utils.run_bass_kernel_spmd(nc, in_maps, core_ids=list(range(B)))
    out = np.stack([res.results[b]["y"] for b in range(B)], axis=0)
    return out.reshape(B, OUT, H, W).astype(np.float32)


if __name__ == "__main__":
    pass


# revision 27
# speedup vs baseline: 1.1701x; 1.1701x over previous
"""CrossModalAttention TRN2 kernel.

Strategy (data-parallel over batch, one batch element per NeuronCore):
  dir a: q from rgb, k/v from pl;  dir b: q from pl, k/v from rgb.
  bf16 datapath (SBUF), fp32 PSUM accumulation.
  Key identity: the K-projection bias cancels in softmax over keys
  (it adds a per-query constant to every score), so K needs no bias.
  Per direction:
    Q  = scale*Wq @ f_q + scale*bq    [128 e, N]  (scale folded on host)
    K  = Wk @ f_k                     [128 e, N]  (no bias)
    VT = (Wv @ f_k)^T                 [N k, 128 e] (v-bias folded into BN shift)
    per q-tile (512 wide):
      S^T_j = K_j^T @ Q_tile          [128 k, 512 q]  per k-chunk j (PSUM f32)
      E_j   = exp(S^T_j)              (ScalarE, bf16 out)
      OT   += VT_j^T @ E_j            [128 e, 512 q]  (PSUM accumulate over j)
      dn    = ones^T @ octsum(E)      [1, 512 q]  (DVE pair/quad/oct tree ->
                                       only 4 denominator matmuls per tile)
      OT_norm = OT * bcast(1/dn)      (recip_approx_fast; bcast via rank-1 MM)
  y = Wp_a @ OT_a + Wp_b @ OT_b ; out = relu(inv*y + shift)  (BN folded, incl.
  v-bias contribution: shift' = beta - mean*inv + inv*(Wp_a@bv_a + Wp_b@bv_b))

Scheduling: PE is the in-order bottleneck engine, so everything that
depends on slow DVE/ACT chains is lag-scheduled: denominator matmuls run
2 groups after their tree-sum is emitted, and each segment's normalize/
projection tail is spilled into the next segment's early groups (hooks).
"""

import sys

sys.path.insert(0, "/opt/trn_rl_repo")

import numpy as np

B = 8
C = 256
E = 128
OUT = 256
H = W = 64
N = H * W
QW = 512
SCALE = float(E) ** -0.5

_CACHE = {}


def _patch_tail_drain(tile_mod, mybir):
    # This walrus build encodes Drain as CTRL_NO_STRUCT with a single
    # sync-wait slot; split the TileContext tail drain's waits across
    # one drain instruction per semaphore.
    if getattr(tile_mod.TileContext, "_drain_patched", False):
        return
    from concourse.vector_clock import ScopedClock

    def _drain_and_barrier(self, tick_clock, wait_clock):
        nc = self.nc
        drain_inst = nc.sync.drain()
        wait_clock.add_sem_waits(
            drain_inst.ins, ScopedClock({None: tick_clock.global_clock})
        )
        si = drain_inst.ins.sync_info
        if si is not None and si.on_wait and len(si.on_wait) > 1:
            waits = list(si.on_wait)
            drain_inst.ins.sync_info = mybir.SyncInfo(
                on_wait=[waits[0]], on_update=list(si.on_update or [])
            )
            for w in waits[1:]:
                d2 = nc.sync.drain()
                d2.ins.sync_info = mybir.SyncInfo(on_wait=[w], on_update=[])
        nc.all_engine_barrier()
        popped = nc._tile_sem_poison_stack.pop()
        assert popped is self._sem_poison
        nc.clear_and_free_semaphores(list(self.sems.allocated().values()))
        nc.all_engine_barrier()

    tile_mod.TileContext._drain_and_barrier = _drain_and_barrier
    tile_mod.TileContext._drain_patched = True


def build_nc(n=N, debug=False):
    """Build the single-core Bass program. n = spatial size (4096 full)."""
    import concourse.bacc as bacc
    import concourse.tile as tile
    from concourse import mybir

    f32 = mybir.dt.float32
    f32r = mybir.dt.float32r
    bf16 = mybir.dt.bfloat16
    AFT = mybir.ActivationFunctionType

    gj = 2                    # k-chunks per S/exp group
    nqt = n // QW             # q tiles per direction
    nkc = n // 128            # k chunks
    ngrp = nkc // gj          # exp groups per segment

    nc = bacc.Bacc(trn_type="TRN2", target_bir_lowering=False, debug=False)

    def din(name, shape, dt_=bf16):
        return nc.dram_tensor(name, shape, dt_, kind="ExternalInput").ap()

    f_a_d = din("f_a", [C, n])        # rgb features (q-side of dir a)
    f_b_d = din("f_b", [C, n])        # pl features
    wq_a_d = din("wq_a", [C, E])      # (scale*W_q_rgb)^T
    wk_a_d = din("wk_a", [C, E])      # W_k_pl^T
    wv_a_d = din("wv_a", [C, E])      # W_v_pl^T
    wq_b_d = din("wq_b", [C, E])      # (scale*W_q_pl)^T
    wk_b_d = din("wk_b", [C, E])      # W_k_rgb^T
    wv_b_d = din("wv_b", [C, E])      # W_v_rgb^T
    wp_d = din("wp", [2 * E, OUT])    # w_proj^T
    bq_a_d = din("bq_a", [E, 1], f32)  # scale * b_q_rgb
    bq_b_d = din("bq_b", [E, 1], f32)  # scale * b_q_pl
    inv_d = din("bn_inv", [OUT, 1], f32)
    shf_d = din("bn_shf", [OUT, 1], f32)
    ones_c_d = din("ones_c", [E, 1], f32)   # f32r for dn matmul
    ones_r_d = din("ones_r", [1, E], f32)   # f32r for bcast matmul
    ident_d = din("ident", [E, E], f32)     # f32r for PE transposes
    y_d = nc.dram_tensor("y", [OUT, n], f32, kind="ExternalOutput").ap()

    with tile.TileContext(nc) as tc:
        with tc.tile_pool(name="const", bufs=1) as const, \
             tc.tile_pool(name="qkv", bufs=1) as qkv, \
             tc.tile_pool(name="pst", bufs=2, space="PSUM") as pst, \
             tc.tile_pool(name="pot", bufs=2, space="PSUM") as pot, \
             tc.tile_pool(name="psh", bufs=1, space="PSUM") as psh:
            # ---- constants (DMA order: critical-path first) ----
            def wload(d, nm, eng):
                t = const.tile([128, 2, E], bf16, name=nm, tag=nm)
                eng.dma_start(t[:], d.rearrange("(c p) e -> p c e", p=128))
                return t

            def vload(d, shape, nm, eng, dt_=f32):
                t = const.tile(shape, dt_, name=nm, tag=nm)
                eng.dma_start(t[:], d)
                return t

            def vload_r(d, shape, nm, eng):
                t = const.tile(shape, f32r, name=nm, tag=nm)
                eng.dma_start(t[:], d.bitcast(f32r))
                return t

            # dir-0 weights on the sync queue (feature pieces follow right
            # behind); dir-1 weights + biases issue from the still-idle
            # scalar queue, after the first cc1 feature piece.
            dum = const.tile([1, 8], f32, name="dum", tag="dum")
            nc.vector.memset(dum[:], 0.0)

            wq = {0: wload(wq_a_d, "wqa", nc.sync)}
            wk = {0: wload(wk_a_d, "wka", nc.sync)}
            wv = {0: wload(wv_a_d, "wva", nc.sync)}

            # ---- per-direction activations (bf16) ----
            q_sb = {d: qkv.tile([128, n], bf16, tag=f"q{d}", name=f"q_sb{d}") for d in (0, 1)}
            k_sb = {d: qkv.tile([128, n], bf16, tag=f"k{d}", name=f"k_sb{d}") for d in (0, 1)}
            vt_sb = {d: qkv.tile([128, n], bf16, tag=f"v{d}", name=f"vt_sb{d}") for d in (0, 1)}

            # 8-deep PSUM slot rotation (all 8 banks) for the projection
            # head: the per-slot eviction latency then never stalls the PE.
            class Slots:
                def __init__(self):
                    self.i = 0
                    self.cur = None

                def get(self):
                    m = self.i % 8
                    self.i += 1
                    if m < 4:
                        if m % 2 == 0:
                            self.cur = pst.tile(
                                [128, gj, QW], f32, tag="st", name="pps")
                        return (self.cur, m % 2)
                    if m < 6:
                        return (pot.tile([128, QW], f32, tag="ot", name="ppo"),
                                None)
                    if m == 6:
                        return (psh.tile([128, QW], f32, tag="sh", name="pph"),
                                None)
                    return (psh.tile([128, QW], f32, tag="dn", name="ppd"),
                            None)

            def sl(slot, s=slice(None)):
                t, h = slot
                return t[:, h, s] if h is not None else t[:, s]

            # ---- feature load + projections (feature pool freed after) ----
            with tc.tile_pool(name="feat", bufs=1) as feat:
                fsb = {
                    name: feat.tile([128, 2, n], bf16, tag=f"f{name}",
                                    name=f"f_{name}")
                    for name in ("a", "b")
                }
                npc = max(1, n // 512)   # small pieces: projections start early
                for pc in range(npc):
                    lo, hi = pc * (n // npc), (pc + 1) * (n // npc)
                    for cc in range(2):
                        for name, dd in (("a", f_a_d), ("b", f_b_d)):
                            # cc0 on sync; first two cc1 pieces on scalar
                            # (both HWDGE, fast issue); later cc1 pieces on
                            # gpsimd (its ~8us engine preamble + ~1us/issue
                            # is fine off the critical path)
                            if cc == 0:
                                eng = nc.sync
                            elif pc == 0:
                                eng = nc.scalar
                            else:
                                eng = nc.gpsimd
                            eng.dma_start(
                                fsb[name][:, cc, lo:hi],
                                dd[cc * 128:(cc + 1) * 128, lo:hi],
                            )
                    if pc == 0:
                        # force the exp_and_others ACT table load now (it
                        # otherwise serializes ~2.7us in front of the first
                        # Q/K eviction), after the cc1-piece issues
                        dum2 = const.tile([1, 8], f32, name="dum2", tag="dum2")
                        nc.scalar.activation(dum2[:], dum[:], AFT.Exp)
                        # dir-1 weights + biases behind the first cc1 piece
                        wq[1] = wload(wq_b_d, "wqb", nc.scalar)
                        wk[1] = wload(wk_b_d, "wkb", nc.scalar)
                        wv[1] = wload(wv_b_d, "wvb", nc.scalar)
                        bq = {0: vload(bq_a_d, [E, 1], "bqa", nc.scalar),
                              1: vload(bq_b_d, [E, 1], "bqb", nc.scalar)}
                # late consts (used mid/late) on gpsimd after the cc1 pieces
                wp = const.tile([128, 2, OUT], bf16, name="wp", tag="wp")
                nc.gpsimd.dma_start(
                    wp[:], wp_d.rearrange("(c p) e -> p c e", p=128))
                binv = const.tile([128, 2, 1], f32, name="binv", tag="binv")
                nc.gpsimd.dma_start(
                    binv[:], inv_d.rearrange("(c p) e -> p c e", p=128))
                bshf = const.tile([128, 2, 1], f32, name="bshf", tag="bshf")
                nc.gpsimd.dma_start(
                    bshf[:], shf_d.rearrange("(c p) e -> p c e", p=128))
                ones_c = vload_r(ones_c_d, [E, 1], "onc", nc.gpsimd)
                ones_r = vload_r(ones_r_d, [1, E], "onr", nc.gpsimd)
                ident = vload_r(ident_d, [E, E], "idt", nc.gpsimd)

                slots = Slots()
                vtmps = {
                    d: feat.tile([128, n], f32r, tag=f"vtmp{d}",
                                 name=f"vtmp{d}")
                    for d in (0, 1)
                }
                # prewarm partition_broadcast's ext-isa library (~6us IRAM
                # load) under the projection head instead of mid-attention
                pbw_s = const.tile([1, 8], f32, name="pbw_s", tag="pbw_s")
                nc.gpsimd.memset(pbw_s[:], 1.0)
                pbw_d = const.tile([128, 8], f32, name="pbw_d", tag="pbw_d")
                nc.gpsimd.partition_broadcast(pbw_d[:], pbw_s[:])
                for nt in range(nqt):
                    for d in (0, 1):
                        fq = fsb["a"] if d == 0 else fsb["b"]
                        fk = fsb["b"] if d == 0 else fsb["a"]
                        for which, wt, src_f in (
                            ("q", wq[d], fq), ("k", wk[d], fk), ("v", wv[d], fk),
                        ):
                            slot = slots.get()
                            ps = sl(slot)
                            for cc in range(2):
                                nc.tensor.matmul(
                                    ps,
                                    wt[:, cc, :],
                                    src_f[:, cc, nt * QW:(nt + 1) * QW],
                                    start=(cc == 0),
                                    stop=(cc == 1),
                                )
                            if which == "q":
                                nc.scalar.activation(
                                    q_sb[d][:, nt * QW:(nt + 1) * QW], ps,
                                    AFT.Identity, bias=bq[d][:],
                                )
                            elif which == "k":
                                nc.scalar.activation(
                                    k_sb[d][:, nt * QW:(nt + 1) * QW], ps,
                                    AFT.Copy,
                                )
                            else:
                                with nc.allow_low_precision(reason="f32r V"):
                                    nc.vector.tensor_copy(
                                        vtmps[d][:, nt * QW:(nt + 1) * QW], ps
                                    )
                # PE transposes after both dirs' projections
                for d in (0, 1):
                    for g in range(nkc // 4):
                        slot = slots.get()
                        for jj in range(4):
                            kc = 4 * g + jj
                            nc.tensor.transpose(
                                sl(slot, slice(jj * 128, (jj + 1) * 128))
                                .bitcast(f32r),
                                vtmps[d][:, kc * 128:(kc + 1) * 128],
                                ident[:],
                            )
                        with nc.allow_low_precision(reason="bf16 VT"):
                            nc.vector.tensor_copy(
                                vt_sb[d][:, g * 512:(g + 1) * 512],
                                sl(slot).bitcast(f32r),
                            )

            # ---- attention + output ----
            with tc.tile_pool(name="sex", bufs=4) as sex, \
                 tc.tile_pool(name="sred", bufs=4) as sred, \
                 tc.tile_pool(name="sot", bufs=3) as sot, \
                 tc.tile_pool(name="smisc", bufs=4) as smisc:

                def emit_S(d, qt, g):
                    """S^T matmuls for one k-chunk group -> st psum tile."""
                    qs = q_sb[d][:, qt * QW:(qt + 1) * QW]
                    st = pst.tile([128, gj, QW], f32, tag="st", name="st")
                    for jj in range(gj):
                        j = gj * g + jj
                        nc.tensor.matmul(
                            st[:, jj, :],
                            k_sb[d][:, j * 128:(j + 1) * 128],
                            qs,
                            start=True, stop=True,
                        )
                    return st

                segs = [(qt, d) for qt in range(nqt) for d in (0, 1)]
                pending = {}          # qt -> {d: osb}

                def emit_body(d, qt, st0, hooks, next_seg):
                    """exp + O/dn accumulation for one (qt, d); st0 is the
                    pre-emitted group-0 S tile. hooks: group -> thunks from
                    the previous segment's tail. At the last group, the NEXT
                    segment's group-0 S is emitted (before this group's O
                    matmuls) so the ScalarE never gaps at the boundary.
                    Returns (ot, dn, dnq, st0_of_next)."""
                    ot = pot.tile([128, QW], f32, tag="ot", name="ot")
                    dn = psh.tile([128, QW], f32, tag="dn", name="dn")
                    st_cur = st0
                    exq = exo_prev = None
                    st_next0 = None
                    dnq = []          # (exoo, j) awaiting lagged dn matmul
                    for g in range(ngrp):
                        if g + 1 < ngrp:
                            st_next = emit_S(d, qt, g + 1)
                        else:
                            st_next = None
                            if next_seg is not None:
                                nqt_, nd_ = next_seg
                                st_next0 = emit_S(nd_, nqt_, 0)
                        ex = sex.tile([128, gj, QW], bf16, tag="ex", name="ex")
                        nc.scalar.activation(ex[:], st_cur[:], AFT.Exp)
                        # denominator tree: pair/quad (bf16) -> oct (f32r)
                        exs = sred.tile([128, QW], bf16, tag="exs", name="exs")
                        with nc.allow_low_precision(reason="bf16 denom"):
                            nc.vector.tensor_add(exs[:], ex[:, 0, :], ex[:, 1, :])
                        for jj in range(gj):
                            j = gj * g + jj
                            nc.tensor.matmul(
                                ot[:],
                                vt_sb[d][:, j * 128:(j + 1) * 128],
                                ex[:, jj, :],
                                start=(j == 0), stop=(j == nkc - 1),
                            )
                        if g % 2 == 0:
                            exq = exs
                        else:
                            exo = sred.tile([128, QW], bf16, tag="exo",
                                            name="exo")
                            with nc.allow_low_precision(reason="bf16 denom"):
                                nc.vector.tensor_add(exo[:], exq[:], exs[:])
                            if g % 4 == 1:
                                exo_prev = exo
                            else:
                                exoo = sred.tile([128, QW], f32r, tag="exoo",
                                                 name="exoo")
                                with nc.allow_low_precision(reason="f32r denom"):
                                    nc.vector.tensor_add(
                                        exoo[:], exo_prev[:], exo[:])
                                dnq.append((exoo, g // 4))
                        # lagged dn matmul: oct j runs at group 4j+5 so the
                        # DVE tree is always done before the PE reaches it
                        if dnq and dnq[0][1] * 4 + 5 <= g:
                            exoo, j = dnq.pop(0)
                            nc.tensor.matmul(
                                dn[0:1, :], ones_c[:], exoo[:],
                                start=(j == 0), stop=(j == 3),
                            )
                        for th in hooks.get(g, ()):
                            th()
                        st_cur = st_next
                    return ot, dn, dnq

                def make_tail_hooks(qt, d, ot, dn):
                    """Thunks for this segment's tail, run in the next
                    segment's early groups."""
                    rcf = smisc.tile([1, QW], f32, tag="rcf", name="rcf")
                    hold = {}

                    def t_recip():    # DVE, at the boundary right after dn3
                        nc.vector.reciprocal_approx_fast(rcf[:], dn[0:1, :])

                    def t_bc():       # GPSIMD: partition broadcast of 1/dn
                        bc_sb = smisc.tile([128, QW], f32, tag="bcs",
                                           name="bcs")
                        hold["bc"] = bc_sb
                        nc.gpsimd.partition_broadcast(bc_sb[:], rcf[:])

                    def t_mul():      # DVE: normalize
                        osb = sot.tile([128, QW], bf16, tag="osb", name="osb")
                        with nc.allow_low_precision(reason="bf16 osb"):
                            nc.vector.tensor_mul(osb[:], ot[:], hold["bc"][:])
                        pending.setdefault(qt, {})[d] = osb

                    return t_recip, t_bc, t_mul

                def make_final_hook(qt, dch):
                    def th():
                        osbs = pending[qt]
                        yp = psh.tile([128, QW], f32, tag="sh", name="yp")
                        for d2 in (0, 1):
                            nc.tensor.matmul(
                                yp[:],
                                wp[:, d2, dch * 128:(dch + 1) * 128],
                                osbs[d2][:],
                                start=(d2 == 0), stop=(d2 == 1),
                            )
                        # BN affine + relu on DVE (keeps ScalarE free for exps)
                        ya = smisc.tile([128, QW], f32, tag="ya", name="ya")
                        nc.vector.tensor_scalar(
                            ya[:], yp[:], binv[:, dch, :], bshf[:, dch, :],
                            mybir.AluOpType.mult, mybir.AluOpType.add,
                        )
                        ysb = smisc.tile([128, QW], f32, tag="ysb", name="ysb")
                        nc.vector.tensor_scalar_max(ysb[:], ya[:], 0.0)
                        nc.sync.dma_start(
                            y_d[dch * 128:(dch + 1) * 128,
                                qt * QW:(qt + 1) * QW],
                            ysb[:],
                        )
                        if dch == 1:
                            pending.pop(qt)
                    return th

                st_next0 = emit_S(segs[0][1], segs[0][0], 0)
                hooks = {}
                final_q = []     # final thunks, drained one per segment
                for i, (qt, d) in enumerate(segs):
                    ot, dn, dnq = emit_body(d, qt, st_next0, hooks)
                    last = i + 1 >= len(segs)
                    if not last:
                        nqt_, nd_ = segs[i + 1]
                        st_next0 = emit_S(nd_, nqt_, 0)
                    # close this segment's denominator: final oct matmul
                    assert len(dnq) == 1
                    exoo, j = dnq.pop(0)
                    nc.tensor.matmul(
                        dn[0:1, :], ones_c[:], exoo[:],
                        start=(j == 0), stop=(j == 3),
                    )
                    t_recip, t_bc, t_mul = make_tail_hooks(qt, d, ot, dn)
                    t_recip()
                    hooks = {1: [t_bc], 3: [t_mul]}
                    if d == 1:
                        final_q.append(make_final_hook(qt, 0))
                        final_q.append(make_final_hook(qt, 1))
                    if final_q:
                        hooks[4] = [final_q.pop(0)]
                    if last:
                        for g in sorted(hooks):
                            for th in hooks[g]:
                                th()
                        for th in final_q:
                            th()
    nc.compile()
    return nc


def _host_prep(inputs, n=N):
    import ml_dtypes
    bf = ml_dtypes.bfloat16

    f_rgb = np.ascontiguousarray(
        inputs["f_rgb"].reshape(B, C, n).astype(bf))
    f_pl = np.ascontiguousarray(
        inputs["f_pl"].reshape(B, C, n).astype(bf))

    def T(w, scale=1.0):
        return np.ascontiguousarray(
            (scale * np.asarray(w, np.float32)).T).astype(bf)

    wp = np.asarray(inputs["w_proj"], np.float32)
    inv = np.asarray(inputs["bn_gamma"], np.float32) / np.sqrt(
        np.asarray(inputs["bn_var"], np.float32) + 1e-5)
    shift = (np.asarray(inputs["bn_beta"], np.float32)
             - np.asarray(inputs["bn_mean"], np.float32) * inv
             + inv * (wp[:, :E] @ np.asarray(inputs["b_v_pl"], np.float32)
                      + wp[:, E:] @ np.asarray(inputs["b_v_rgb"], np.float32)))

    shared = {
        "wq_a": T(inputs["w_q_rgb"], SCALE),
        "wk_a": T(inputs["w_k_pl"]),
        "wv_a": T(inputs["w_v_pl"]),
        "wq_b": T(inputs["w_q_pl"], SCALE),
        "wk_b": T(inputs["w_k_rgb"]),
        "wv_b": T(inputs["w_v_rgb"]),
        "wp": T(wp),
        "bq_a": (SCALE * np.asarray(inputs["b_q_rgb"], np.float32))
        .reshape(E, 1).copy(),
        "bq_b": (SCALE * np.asarray(inputs["b_q_pl"], np.float32))
        .reshape(E, 1).copy(),
        "bn_inv": inv.reshape(OUT, 1).copy(),
        "bn_shf": shift.reshape(OUT, 1).copy(),
        "ones_c": np.ones((E, 1), np.float32),
        "ones_r": np.ones((1, E), np.float32),
        "ident": np.eye(E, dtype=np.float32),
    }
    in_maps = []
    for b in range(B):
        m = dict(shared)
        m["f_a"] = f_rgb[b]
        m["f_b"] = f_pl[b]
        in_maps.append(m)
    return in_maps


def kernel(**inputs):
    from concourse import bass_utils

    if "nc" not in _CACHE:
        _CACHE["nc"] = build_nc()
    nc = _CACHE["nc"]
    in_maps = _host_prep(inputs)
    res = bass_utils.run_bass_kernel_spmd(nc, in_maps, core_ids=list(range(B)))
    out = np.stack([res.results[b]["y"] for b in range(B)], axis=0)
    return out.reshape(B, OUT, H, W).astype(np.float32)


if __name__ == "__main__":
    pass
